# revision 7
# baseline (speedup 1.0000x reference)
"""Trainium2 Bass kernel for nn_EquivariantFeedForward.

Reference computation (per batch element b, token n):
    pm  = einsum('ndm,de->nem', x, pre)         per degree
    inv = ||pm||_2 over m                        -> (n, d)
    hid = silu([h | inv] @ w1 + b1)
    out = hid @ w2 + b2 ; m1, m2 = out[:, :D], out[:, D:]
    p   = einsum('ndm,de->nem', x, proj)
    y_x = x + p * m2[..., None]
    y_h = h + m1_deg1 + m1_deg2

Sharding: data-parallel over batch (B=8 -> one batch element per core),
weights replicated. All matmuls run as float32r (full fp32 bits through
the PE fast path). Activations are kept channel-major ([feature_part,
token_free]) for the matmuls via PE transposes; the proj einsum is
emitted token-major so the gating multiply and residual add happen in
the DMA-friendly token-major layout.
"""

import numpy as np

import concourse.bass as bass
import concourse.bacc as bacc
import concourse.tile as tile
from concourse import mybir
from concourse.bass_utils import run_bass_kernel_spmd
from concourse.masks import make_identity

F32 = mybir.dt.float32
F32R = mybir.dt.float32r
AF = mybir.ActivationFunctionType

B, D = 8, 256
M1, M2 = 3, 5
HID = 1024
K2D = 2 * D
T = 256          # tokens per tile
GROUP = 2        # tiles per group (sqrt table-switch batching)


def r(ap):
    return ap.bitcast(F32R)


def build_kernel(n_tokens=4096):
    """Build the per-core Bass module (one batch element per core)."""
    nc = bacc.Bacc("TRN2", target_bir_lowering=False, debug=False, num_devices=8)

    h = nc.declare_dram_parameter("h", [n_tokens, D], F32, isOutput=False).ap()
    x1 = nc.declare_dram_parameter("x1", [n_tokens, D, M1], F32, isOutput=False).ap()
    x2 = nc.declare_dram_parameter("x2", [n_tokens, D, M2], F32, isOutput=False).ap()
    wd = {}
    for name, shape in [
        ("pre1", [D, D]), ("pre2", [D, D]), ("proj1", [D, D]), ("proj2", [D, D]),
        ("w1_0", [K2D, HID]), ("w2_0", [HID, K2D]),
        ("w1_1", [K2D, HID]), ("w2_1", [HID, K2D]),
        ("b1_0", [HID]), ("b2_0", [K2D]), ("b1_1", [HID]), ("b2_1", [K2D]),
    ]:
        wd[name] = nc.declare_dram_parameter(name, shape, F32, isOutput=False).ap()
    ho = nc.declare_dram_parameter("ho", [n_tokens, D], F32, isOutput=True).ap()
    y1 = nc.declare_dram_parameter("y1", [n_tokens, D, M1], F32, isOutput=True).ap()
    y2 = nc.declare_dram_parameter("y2", [n_tokens, D, M2], F32, isOutput=True).ap()

    n_tiles = n_tokens // T
    n_groups = n_tiles // GROUP
    MS = (M1, M2)

    with tile.TileContext(nc) as tc:
        import contextlib
        ctx = contextlib.ExitStack()
        with ctx:
            singles = ctx.enter_context(tc.tile_pool(name="singles", bufs=1))
            # ---- weights, resident in SBUF ----
            identity = singles.tile([128, 128], F32)
            make_identity(nc, identity[:])
            idf = identity[:]

            wstage_p = ctx.enter_context(tc.tile_pool(name="wstage", bufs=1))

            def load_w(apname, rows, cols):
                tiles = []
                for c in range(rows // 128):
                    stg = wstage_p.tile([128, cols], F32, name="wstage", tag="wstage")
                    nc.sync.dma_start(out=stg[:], in_=wd[apname][c * 128:(c + 1) * 128, :])
                    t_ = singles.tile([128, cols], F32R, name=f"w_{apname}_{c}", tag=f"w_{apname}_{c}")
                    nc.vector.tensor_copy(t_[:], stg[:])
                    tiles.append(t_)
                return tiles

            wpre = [load_w("pre1", D, D), load_w("pre2", D, D)]
            wproj = [load_w("proj1", D, D), load_w("proj2", D, D)]
            w1 = [load_w("w1_0", K2D, HID), load_w("w1_1", K2D, HID)]
            w2 = [load_w("w2_0", HID, K2D), load_w("w2_1", HID, K2D)]

            def load_bias(apname, n):
                t_ = singles.tile([128, n // 128], F32, name=f"b_{apname}", tag=f"b_{apname}")
                nc.sync.dma_start(
                    out=t_[:], in_=wd[apname].rearrange("(c p) -> p c", p=128))
                return t_

            b1sb = [load_bias("b1_0", HID), load_bias("b1_1", HID)]
            b2sb = [load_bias("b2_0", K2D), load_bias("b2_1", K2D)]
            # combined m1 bias: b2_0[:D] + b2_1[:D]  (as [128, 2] chunks)
            b2m1 = singles.tile([128, 2], F32)
            nc.vector.tensor_add(b2m1[:], b2sb[0][:, 0:2], b2sb[1][:, 0:2])

            # ---- pools ----
            xt_p = [
                ctx.enter_context(tc.tile_pool(name="xt1", bufs=3)),
                ctx.enter_context(tc.tile_pool(name="xt2", bufs=2)),
            ]
            ht_p = ctx.enter_context(tc.tile_pool(name="ht", bufs=2))
            xc_p = [
                ctx.enter_context(tc.tile_pool(name="xc1", bufs=3)),
                ctx.enter_context(tc.tile_pool(name="xc2", bufs=2)),
            ]
            hc_p = ctx.enter_context(tc.tile_pool(name="hc", bufs=3))
            pmsq_p = ctx.enter_context(tc.tile_pool(name="pmsq", bufs=2))
            inv2_p = ctx.enter_context(tc.tile_pool(name="inv2", bufs=2))
            inv_p = ctx.enter_context(tc.tile_pool(name="inv", bufs=8))
            hid_p = ctx.enter_context(tc.tile_pool(name="hids", bufs=9))
            sm_p = ctx.enter_context(tc.tile_pool(name="small", bufs=2))
            mod_p = ctx.enter_context(tc.tile_pool(name="modtmp", bufs=2))
            yh_p = ctx.enter_context(tc.tile_pool(name="yh", bufs=2))

            ps_work = ctx.enter_context(
                tc.tile_pool(name="ps_work", bufs=6, space="PSUM"))
            ps_tp = ctx.enter_context(
                tc.tile_pool(name="ps_tp", bufs=2, space="PSUM"))

            TOK = {}   # tile-index -> dict of live tiles

            def phase_load(t):
                """DMA + PE transposes -> channel-major tiles; pre einsums -> inv2."""
                st = {}
                tok0 = t * T
                ht = ht_p.tile([128, 2, D], F32, name="ht", tag="ht")
                xt = [xt_p[d].tile([128, 2, D * MS[d]], F32, name=f"xt{d}", tag=f"xt{d}")
                      for d in range(2)]
                for tb in range(2):
                    sl = slice(tok0 + tb * 128, tok0 + (tb + 1) * 128)
                    nc.sync.dma_start(out=ht[:, tb], in_=h[sl, :])
                    nc.sync.dma_start(out=xt[0][:, tb], in_=x1[sl].rearrange("n d m -> n (d m)"))
                    nc.sync.dma_start(out=xt[1][:, tb], in_=x2[sl].rearrange("n d m -> n (d m)"))
                st["ht"], st["xt"] = ht, xt

                # h -> channel major
                hc = [hc_p.tile([128, T], F32R, name=f"hc{dc}", tag=f"hc{dc}") for dc in range(2)]
                for dc in range(2):
                    ps = ps_tp.tile([128, T], F32, name="tp", tag="tp")
                    for tb in range(2):
                        nc.tensor.transpose(
                            ps[:, tb * 128:(tb + 1) * 128],
                            ht[:, tb, dc * 128:(dc + 1) * 128], idf)
                    nc.vector.tensor_copy(hc[dc][:], ps[:])
                st["hc"] = hc

                # x -> channel major
                xc = [xc_p[d].tile([128, 2, T, MS[d]], F32R, name=f"xc{d}", tag=f"xc{d}")
                      for d in range(2)]
                for d in range(2):
                    xtv = [xt[d][:, tb].rearrange("p (d m) -> p d m", m=MS[d])
                           for tb in range(2)]
                    for dc in range(2):
                        for mi in range(MS[d]):
                            ps = ps_tp.tile([128, T], F32, name="tp", tag="tp")
                            for tb in range(2):
                                nc.tensor.transpose(
                                    ps[:, tb * 128:(tb + 1) * 128],
                                    xtv[tb][:, dc * 128:(dc + 1) * 128, mi], idf)
                            nc.vector.tensor_copy(
                                xc[d][:, dc, :, mi], ps[:])
                st["xc"] = xc

                # pre einsums -> pm -> square -> reduce -> inv2
                inv2 = []
                for d in range(2):
                    inv2_d = []
                    for ec in range(2):
                        pmsq = pmsq_p.tile([128, T, MS[d]], F32, name="pmsq", tag="pmsq")
                        for mi in range(MS[d]):
                            pm = ps_work.tile([128, T], F32, name="work", tag="work")
                            for dc in range(2):
                                nc.tensor.matmul(
                                    pm[:],
                                    wpre[d][dc][:, ec * 128:(ec + 1) * 128],
                                    xc[d][:, dc, :, mi],
                                    start=(dc == 0), stop=(dc == 1))
                            nc.scalar.square(pmsq[:, :, mi], pm[:])
                        iv2 = inv2_p.tile([128, T], F32, name="inv2", tag="inv2")
                        nc.vector.tensor_reduce(
                            iv2[:], pmsq[:], axis=mybir.AxisListType.X,
                            op=mybir.AluOpType.add)
                        inv2_d.append(iv2)
                    inv2.append(inv2_d)
                st["inv2"] = inv2
                TOK[t] = st

            def phase_sqrt(t):
                st = TOK[t]
                st["inv"] = []
                for d in range(2):
                    inv_d = []
                    for ec in range(2):
                        iv = inv_p.tile([128, T], F32R, name="inv", tag="inv")
                        nc.scalar.activation(iv[:], st["inv2"][d][ec][:], AF.Sqrt)
                        inv_d.append(iv)
                    st["inv"].append(inv_d)

            def phase_mlp(t):
                """Both degree MLPs; leaves m2Ts (token-major gates) and yh."""
                st = TOK[t]
                m1sb = [sm_p.tile([128, T], F32, name=f"m1sb{oc}", tag=f"m1sb{oc}") for oc in range(2)]
                m2Ts = []
                for d in range(2):
                    rhs = [st["hc"][0], st["hc"][1],
                           st["inv"][d][0], st["inv"][d][1]]
                    hids = []
                    for mc in range(HID // 128):
                        psh = ps_work.tile([128, T], F32, name="work", tag="work")
                        for kc in range(4):
                            nc.tensor.matmul(
                                psh[:],
                                w1[d][kc][:, mc * 128:(mc + 1) * 128],
                                rhs[kc][:],
                                start=(kc == 0), stop=(kc == 3))
                        hs = hid_p.tile([128, T], F32R, name="hid", tag="hid")
                        nc.scalar.activation(
                            hs[:], psh[:], AF.Silu, bias=b1sb[d][:, mc:mc + 1])
                        hids.append(hs)
                    # out = hid @ w2 (+b2): oc 0..1 -> m1, oc 2..3 -> m2
                    m2sb_d = []
                    for oc in range(4):
                        pso = ps_work.tile([128, T], F32, name="work", tag="work")
                        for kc in range(8):
                            nc.tensor.matmul(
                                pso[:],
                                w2[d][kc][:, oc * 128:(oc + 1) * 128],
                                hids[kc][:],
                                start=(kc == 0), stop=(kc == 7))
                        if oc < 2:
                            if d == 0:
                                nc.scalar.activation(
                                    m1sb[oc][:], pso[:], AF.Identity,
                                    bias=b2m1[:, oc:oc + 1])
                            else:
                                nc.vector.tensor_add(m1sb[oc][:], m1sb[oc][:], pso[:])
                        else:
                            ms = sm_p.tile([128, T], F32, name=f"m2sb{oc - 2}", tag=f"m2sb{oc - 2}")
                            nc.scalar.activation(
                                ms[:], pso[:], AF.Identity,
                                bias=b2sb[d][:, oc:oc + 1])
                            m2sb_d.append(ms)
                    # transpose m2 -> token major
                    m2Ts_d = []
                    for tb in range(2):
                        ps = ps_tp.tile([128, T], F32, name="tp", tag="tp")
                        for ocm in range(2):
                            nc.tensor.transpose(
                                ps[:, ocm * 128:(ocm + 1) * 128],
                                m2sb_d[ocm][:, tb * 128:(tb + 1) * 128], idf)
                        mt = sm_p.tile([128, T], F32, name=f"m2T{tb}", tag=f"m2T{tb}")
                        nc.vector.tensor_copy(mt[:], ps[:])
                        m2Ts_d.append(mt)
                    m2Ts.append(m2Ts_d)
                st["m2Ts"] = m2Ts

                # h residual: yh = ht + (m1a + m1b).T
                tok0 = t * T
                for tb in range(2):
                    ps = ps_tp.tile([128, T], F32, name="tp", tag="tp")
                    for oc in range(2):
                        nc.tensor.transpose(
                            ps[:, oc * 128:(oc + 1) * 128],
                            m1sb[oc][:, tb * 128:(tb + 1) * 128], idf)
                    yh = yh_p.tile([128, D], F32, name="yh", tag="yh")
                    nc.vector.tensor_add(yh[:], ps[:], st["ht"][:, tb])
                    nc.sync.dma_start(
                        out=ho[tok0 + tb * 128: tok0 + (tb + 1) * 128, :], in_=yh[:])

            def phase_proj(t):
                """proj einsum token-major, gate by m2T, add into xt, store."""
                st = TOK[t]
                tok0 = t * T
                for d in range(2):
                    for tb in range(2):
                        xtv = st["xt"][d][:, tb].rearrange("p (d m) -> p d m", m=MS[d])
                        for mi in range(MS[d]):
                            psp = ps_work.tile([128, D], F32, name="work", tag="work")
                            for dc in range(2):
                                nc.tensor.matmul(
                                    psp[:],
                                    st["xc"][d][:, dc, tb * 128:(tb + 1) * 128, mi],
                                    wproj[d][dc][:],
                                    start=(dc == 0), stop=(dc == 1))
                            mt = mod_p.tile([128, D], F32, name="modtmp", tag="modtmp")
                            nc.vector.tensor_mul(mt[:], psp[:], st["m2Ts"][d][tb][:])
                            nc.gpsimd.tensor_add(xtv[:, :, mi], xtv[:, :, mi], mt[:])
                        out_ap = (y1, y2)[d]
                        sl = slice(tok0 + tb * 128, tok0 + (tb + 1) * 128)
                        nc.sync.dma_start(
                            out=out_ap[sl].rearrange("n d m -> n (d m)"),
                            in_=st["xt"][d][:, tb])

            for g in range(n_groups):
                ts = [g * GROUP + i for i in range(GROUP)]
                for t in ts:
                    phase_load(t)
                for t in ts:
                    phase_sqrt(t)
                for t in ts:
                    phase_mlp(t)
                    phase_proj(t)
                for t in ts:
                    del TOK[t]

    nc.compile()
    return nc


_NC_CACHE = {}


def _get_nc(n_tokens):
    if n_tokens not in _NC_CACHE:
        _NC_CACHE[n_tokens] = build_kernel(n_tokens)
    return _NC_CACHE[n_tokens]


def kernel(h, x1, x2, proj1, proj2, pre1, pre2,
           w1_0, b1_0, w2_0, b2_0, w1_1, b1_1, w2_1, b2_1):
    h = np.asarray(h, np.float32)
    x1 = np.asarray(x1, np.float32)
    x2 = np.asarray(x2, np.float32)
    n_tokens = h.shape[1]
    nc = _get_nc(n_tokens)
    weights = dict(
        pre1=np.asarray(pre1, np.float32), pre2=np.asarray(pre2, np.float32),
        proj1=np.asarray(proj1, np.float32), proj2=np.asarray(proj2, np.float32),
        w1_0=np.asarray(w1_0, np.float32), b1_0=np.asarray(b1_0, np.float32),
        w2_0=np.asarray(w2_0, np.float32), b2_0=np.asarray(b2_0, np.float32),
        w1_1=np.asarray(w1_1, np.float32), b1_1=np.asarray(b1_1, np.float32),
        w2_1=np.asarray(w2_1, np.float32), b2_1=np.asarray(b2_1, np.float32),
    )
    n_cores = h.shape[0]
    in_maps = [dict(h=h[c], x1=x1[c], x2=x2[c], **weights) for c in range(n_cores)]
    res = run_bass_kernel_spmd(nc, in_maps, list(range(n_cores)))
    h_new = np.stack([res.results[c]["ho"] for c in range(n_cores)])
    y1 = np.stack([res.results[c]["y1"] for c in range(n_cores)])
    y2 = np.stack([res.results[c]["y2"] for c in range(n_cores)])
    return h_new, y1, y2


if __name__ == "__main__":
    rng = np.random.default_rng(0)
    nt = 512
    inputs = dict(
        h=rng.standard_normal((1, nt, D), dtype=np.float32),
        x1=rng.standard_normal((1, nt, D, M1), dtype=np.float32),
        x2=rng.standard_normal((1, nt, D, M2), dtype=np.float32),
        pre1=rng.standard_normal((D, D), dtype=np.float32),
        pre2=rng.standard_normal((D, D), dtype=np.float32),
        proj1=rng.standard_normal((D, D), dtype=np.float32),
        proj2=rng.standard_normal((D, D), dtype=np.float32),
        w1_0=(rng.standard_normal((K2D, HID)) * 0.02).astype(np.float32),
        b1_0=np.zeros(HID, np.float32),
        w2_0=(rng.standard_normal((HID, K2D)) * 0.02).astype(np.float32),
        b2_0=np.zeros(K2D, np.float32),
        w1_1=(rng.standard_normal((K2D, HID)) * 0.02).astype(np.float32),
        b1_1=np.zeros(HID, np.float32),
        w2_1=(rng.standard_normal((HID, K2D)) * 0.02).astype(np.float32),
        b2_1=np.zeros(K2D, np.float32),
    )

    def ref_one(h, x, pre, proj, w1, b1, w2, b2):
        pm = np.einsum('ndm,de->nem', x, pre)
        inv = np.sqrt((pm * pm).sum(-1))
        mi = np.concatenate([h, inv], -1)
        z = mi @ w1 + b1
        hid = z / (1 + np.exp(-z))
        out = hid @ w2 + b2
        m1, m2 = out[:, :D], out[:, D:]
        p = np.einsum('ndm,de->nem', x, proj)
        return m1, p * m2[:, :, None]

    m1a, mod1 = ref_one(inputs['h'][0], inputs['x1'][0], inputs['pre1'],
                        inputs['proj1'], inputs['w1_0'], inputs['b1_0'],
                        inputs['w2_0'], inputs['b2_0'])
    m1b, mod2 = ref_one(inputs['h'][0], inputs['x2'][0], inputs['pre2'],
                        inputs['proj2'], inputs['w1_1'], inputs['b1_1'],
                        inputs['w2_1'], inputs['b2_1'])
    eh = inputs['h'][0] + m1a + m1b
    e1 = inputs['x1'][0] + mod1
    e2 = inputs['x2'][0] + mod2

    ho_, y1_, y2_ = kernel(**inputs)
    for name, got, want in [("h", ho_[0], eh), ("y1", y1_[0], e1), ("y2", y2_[0], e2)]:
        err = np.abs(got - want).max()
        rel = np.linalg.norm(got - want) / np.linalg.norm(want)
        print(f"{name}: absmax {err:.3e} rel {rel:.3e}")


# revision 34
# speedup vs baseline: 4.5184x; 4.5184x over previous
"""Trainium2 Bass kernel for nn_EquivariantFeedForward.

Reference computation (per batch element b, token n):
    pm  = einsum('ndm,de->nem', x, pre)         per degree
    inv = ||pm||_2 over m                        -> (n, d)
    hid = silu([h | inv] @ w1 + b1)
    out = hid @ w2 + b2 ; m1, m2 = out[:, :D], out[:, D:]
    p   = einsum('ndm,de->nem', x, proj)
    y_x = x + p * m2[..., None]
    y_h = h + m1_deg1 + m1_deg2

Sharding: data-parallel over batch (B=8 -> one batch element per core),
weights replicated. All matmuls run as float32r (full fp32 bits through
the PE fast path). Activations are kept channel-major ([feature_part,
token_free]) for the matmuls via PE transposes; the proj einsum is
emitted token-major so the gating multiply and residual add happen in
the DMA-friendly token-major layout.
"""

import numpy as np

import concourse.bass as bass
import concourse.bacc as bacc
import concourse.tile as tile
from concourse import mybir
from concourse.bass_utils import run_bass_kernel_spmd
from concourse.masks import make_identity

F32 = mybir.dt.float32
F32R = mybir.dt.float32r
AF = mybir.ActivationFunctionType

B, D = 8, 256
M1, M2 = 3, 5
HID = 1024
K2D = 2 * D
T = 256          # tokens per tile
GROUP = 2        # tiles per group (sqrt table-switch batching)


def r(ap):
    return ap.bitcast(F32R)


def build_kernel(n_tokens=4096):
    """Build the per-core Bass module (one batch element per core)."""
    nc = bacc.Bacc("TRN2", target_bir_lowering=False, debug=False, num_devices=8)

    h = nc.declare_dram_parameter("h", [n_tokens, D], F32R, isOutput=False).ap()
    x1 = nc.declare_dram_parameter("x1", [n_tokens, D, M1], F32R, isOutput=False).ap()
    x2 = nc.declare_dram_parameter("x2", [n_tokens, D, M2], F32R, isOutput=False).ap()
    wd = {}
    for name, shape in [
        ("pre1", [D, D]), ("pre2", [D, D]), ("proj1", [D, D]), ("proj2", [D, D]),
        ("w1_0", [K2D, HID]), ("w2_0", [HID, K2D]),
        ("w1_1", [K2D, HID]), ("w2_1", [HID, K2D]),
        ("b1_0", [HID]), ("b2_0", [K2D]), ("b1_1", [HID]), ("b2_1", [K2D]),
    ]:
        wdt = F32 if name.startswith("b") else F32R
        wd[name] = nc.declare_dram_parameter(name, shape, wdt, isOutput=False).ap()
    ho = nc.declare_dram_parameter("ho", [n_tokens, D], F32, isOutput=True).ap()
    y1 = nc.declare_dram_parameter("y1", [n_tokens, D, M1], F32, isOutput=True).ap()
    y2 = nc.declare_dram_parameter("y2", [n_tokens, D, M2], F32, isOutput=True).ap()

    n_tiles = n_tokens // T
    n_groups = n_tiles // GROUP
    MS = (M1, M2)

    with tile.TileContext(nc) as tc:
        import contextlib
        ctx = contextlib.ExitStack()
        with ctx:
            singles = ctx.enter_context(tc.tile_pool(name="singles", bufs=1))
            # ---- weights, resident in SBUF ----
            identity = singles.tile([128, 128], F32R)
            idf = identity[:]

            # ---- pools ----
            xt_p = [
                ctx.enter_context(tc.tile_pool(name="xt1", bufs=3)),
                ctx.enter_context(tc.tile_pool(name="xt2", bufs=2)),
            ]
            ht_p = ctx.enter_context(tc.tile_pool(name="ht", bufs=3))
            xc_p = [
                ctx.enter_context(tc.tile_pool(name="xc1", bufs=3)),
                ctx.enter_context(tc.tile_pool(name="xc2", bufs=2)),
            ]
            hc_p = ctx.enter_context(tc.tile_pool(name="hc", bufs=3))
            pmsq_p = ctx.enter_context(tc.tile_pool(name="pmsq", bufs=2))
            inv2_p = ctx.enter_context(tc.tile_pool(name="inv2", bufs=8))
            inv_p = ctx.enter_context(tc.tile_pool(name="inv", bufs=8))
            hid_p = ctx.enter_context(tc.tile_pool(name="hids", bufs=8))
            sm_p = ctx.enter_context(tc.tile_pool(name="small", bufs=2))
            mod_p = ctx.enter_context(tc.tile_pool(name="modtmp", bufs=2))
            yh_p = ctx.enter_context(tc.tile_pool(name="yh", bufs=2))

            ps_work = ctx.enter_context(
                tc.tile_pool(name="ps_work", bufs=5, space="PSUM"))
            ps_tp = ctx.enter_context(
                tc.tile_pool(name="ps_tp", bufs=3, space="PSUM"))

            identity_f = mod_p.tile([128, 128], F32, name="modtmp", tag="modtmp")
            make_identity(nc, identity_f[:])
            nc.vector.tensor_copy(identity[:], identity_f[:])

            def load_w(apname, rows, cols):
                tiles = []
                for c in range(rows // 128):
                    t_ = singles.tile([128, cols], F32R, name=f"w_{apname}_{c}", tag=f"w_{apname}_{c}")
                    nc.scalar.dma_start(out=t_[:], in_=wd[apname][c * 128:(c + 1) * 128, :])
                    tiles.append(t_)
                return tiles

            wpre = [load_w("pre1", D, D), load_w("pre2", D, D)]
            wproj = [load_w("proj1", D, D), load_w("proj2", D, D)]
            w1 = [load_w("w1_0", K2D, HID), load_w("w1_1", K2D, HID)]
            w2 = [load_w("w2_0", HID, K2D), load_w("w2_1", HID, K2D)]

            def load_bias(apname, n):
                t_ = singles.tile([128, n // 128], F32, name=f"b_{apname}", tag=f"b_{apname}")
                nc.scalar.dma_start(
                    out=t_[:], in_=wd[apname].rearrange("(c p) -> p c", p=128))
                return t_

            b1sb = [load_bias("b1_0", HID), load_bias("b1_1", HID)]
            b2sb = [load_bias("b2_0", K2D), load_bias("b2_1", K2D)]
            # combined m1 bias: b2_0[:D] + b2_1[:D]  (as [128, 2] chunks)
            b2m1 = singles.tile([128, 2], F32)
            nc.vector.tensor_add(b2m1[:], b2sb[0][:, 0:2], b2sb[1][:, 0:2])
            act_scratch = singles.tile([128, 1], F32)
            nc.vector.memset(act_scratch[:], 0.0)


            TOK = {}   # tile-index -> dict of live tiles

            def load_d0(t):
                """DMA h/x1 + transposes + pre1 einsum for tile t."""
                st = {}
                tok0 = t * T
                ht = ht_p.tile([128, 2, D], F32R, name="ht", tag="ht")
                xt0 = xt_p[0].tile([128, 2, D * M1], F32R, name="xt0", tag="xt0")
                for tb in range(2):
                    sl = slice(tok0 + tb * 128, tok0 + (tb + 1) * 128)
                    nc.sync.dma_start(out=ht[:, tb], in_=h[sl, :])
                    nc.sync.dma_start(out=xt0[:, tb], in_=x1[sl].rearrange("n d m -> n (d m)"))
                st["ht"], st["xt"] = ht, [xt0, None]

                hc = [hc_p.tile([128, T], F32R, name=f"hc{dc}", tag=f"hc{dc}") for dc in range(2)]
                for dc in range(2):
                    ps = ps_tp.tile([128, T], F32R, name="tp", tag="tp")
                    for tb in range(2):
                        nc.tensor.transpose(
                            ps[:, tb * 128:(tb + 1) * 128],
                            ht[:, tb, dc * 128:(dc + 1) * 128], idf)
                    nc.vector.tensor_copy(hc[dc][:], ps[:])
                st["hc"] = hc
                st["xc"] = [None, None]
                st["inv2"] = [None, None]
                TOK[t] = st
                yield
                yield from _load_deg(t, 0)

            def load_d1(t):
                st = TOK[t]
                tok0 = t * T
                xt1_ = xt_p[1].tile([128, 2, D * M2], F32R, name="xt1", tag="xt1")
                for tb in range(2):
                    sl = slice(tok0 + tb * 128, tok0 + (tb + 1) * 128)
                    nc.sync.dma_start(out=xt1_[:, tb], in_=x2[sl].rearrange("n d m -> n (d m)"))
                st["xt"][1] = xt1_
                yield from _load_deg(t, 1)

            def _load_deg(t, d):
                """transposes + pre einsum + squares + reduce for degree d."""
                st = TOK[t]
                xt = st["xt"][d]
                xc = xc_p[d].tile([128, 2, T, MS[d]], F32R, name=f"xc{d}", tag=f"xc{d}")
                st["xc"][d] = xc
                xtv = [xt[:, tb].rearrange("p (d m) -> p d m", m=MS[d])
                       for tb in range(2)]
                pmsq = [pmsq_p.tile([128, T, MS[d]], F32, name="pmsq", tag="pmsq")
                        for _ in range(2)]
                for mi in range(MS[d]):
                    for dc in range(2):
                        ps = ps_tp.tile([128, T], F32R, name="tp", tag="tp")
                        for tb in range(2):
                            nc.tensor.transpose(
                                ps[:, tb * 128:(tb + 1) * 128],
                                xtv[tb][:, dc * 128:(dc + 1) * 128, mi], idf)
                        nc.vector.tensor_copy(xc[:, dc, :, mi], ps[:])
                    for ec in range(2):
                        pm = ps_work.tile([128, T], F32, name="work", tag="work")
                        for dc in range(2):
                            nc.tensor.matmul(
                                pm[:],
                                wpre[d][dc][:, ec * 128:(ec + 1) * 128],
                                xc[:, dc, :, mi],
                                start=(dc == 0), stop=(dc == 1))
                        nc.scalar.square(pmsq[ec][:, :, mi], pm[:])
                    yield
                inv2_d = []
                for ec in range(2):
                    iv2 = inv2_p.tile([128, T], F32, name="inv2", tag="inv2")
                    nc.vector.tensor_reduce(
                        iv2[:], pmsq[ec][:], axis=mybir.AxisListType.X,
                        op=mybir.AluOpType.add)
                    inv2_d.append(iv2)
                st["inv2"][d] = inv2_d
                yield

            def phase_sqrt(t):
                st = TOK[t]
                st["inv"] = []
                for d in range(2):
                    inv_d = []
                    for ec in range(2):
                        iv = inv_p.tile([128, T], F32R, name="inv", tag="inv")
                        nc.scalar.activation(iv[:], st["inv2"][d][ec][:], AF.Sqrt)
                        inv_d.append(iv)
                    st["inv"].append(inv_d)

            def act_preload_silu(dep):
                # data-dep on `dep` pins this AFTER the sqrts so the silu
                # table load lands in the ACT slack window, not before them
                nc.scalar.activation(act_scratch[:], dep[:, 0:1], AF.Silu)

            def act_preload_sqrt(dep):
                # pinned after the first squares: sqrt table loads while the
                # remaining squares still run (square is in both table sets)
                nc.scalar.activation(act_scratch[:], dep[:, 0:1], AF.Sqrt)

            def phase_mlp(t):
                """Both degree MLPs; leaves m2Ts (token-major gates) and yh."""
                st = TOK[t]
                m1sb = [sm_p.tile([128, T], F32R, name="m1sb", tag="m1sb", bufs=3) for oc in range(2)]
                m2Ts = []
                for d in range(2):
                    rhs = [st["hc"][0], st["hc"][1],
                           st["inv"][d][0], st["inv"][d][1]]
                    hids = []
                    for mc in range(HID // 128):
                        psh = ps_work.tile([128, T], F32, name="work", tag="work")
                        for kc in range(4):
                            nc.tensor.matmul(
                                psh[:],
                                w1[d][kc][:, mc * 128:(mc + 1) * 128],
                                rhs[kc][:],
                                start=(kc == 0), stop=(kc == 3))
                        hs = hid_p.tile([128, T], F32R, name="hid", tag="hid")
                        nc.scalar.activation(
                            hs[:], psh[:], AF.Silu, bias=b1sb[d][:, mc:mc + 1])
                        hids.append(hs)
                        if mc % 2 == 1:
                            yield
                    # out = hid @ w2 (+b2): oc 0..1 -> m1, oc 2..3 -> m2
                    m2sb_d = []
                    for oc in range(4):
                        pso = ps_work.tile([128, T], F32, name="work", tag="work")
                        for kc in range(8):
                            nc.tensor.matmul(
                                pso[:],
                                w2[d][kc][:, oc * 128:(oc + 1) * 128],
                                hids[kc][:],
                                start=(kc == 0), stop=(kc == 7))
                        yield
                        if oc < 2:
                            if d == 0:
                                nc.scalar.activation(
                                    m1sb[oc][:], pso[:], AF.Identity,
                                    bias=b2m1[:, oc:oc + 1])
                            else:
                                nc.vector.tensor_add(m1sb[oc][:], m1sb[oc][:], pso[:])
                        else:
                            ms = sm_p.tile([128, T], F32R, name="m2sb", tag="m2sb")
                            nc.scalar.activation(
                                ms[:], pso[:], AF.Identity,
                                bias=b2sb[d][:, oc:oc + 1])
                            m2sb_d.append(ms)
                    # transpose m2 -> token major
                    m2Ts_d = []
                    for tb in range(2):
                        ps = ps_tp.tile([128, T], F32R, name="tp", tag="tp")
                        for ocm in range(2):
                            nc.tensor.transpose(
                                ps[:, ocm * 128:(ocm + 1) * 128],
                                m2sb_d[ocm][:, tb * 128:(tb + 1) * 128], idf)
                        mt = sm_p.tile([128, T], F32, name=f"m2T{tb}", tag=f"m2T{tb}")
                        nc.vector.tensor_copy(mt[:], ps[:].bitcast(F32))
                        m2Ts_d.append(mt)
                    m2Ts.append(m2Ts_d)
                st["m2Ts"] = m2Ts

                # h residual: yh = ht + (m1a + m1b).T
                tok0 = t * T
                for tb in range(2):
                    ps = ps_tp.tile([128, T], F32R, name="tp", tag="tp")
                    for oc in range(2):
                        nc.tensor.transpose(
                            ps[:, oc * 128:(oc + 1) * 128],
                            m1sb[oc][:, tb * 128:(tb + 1) * 128], idf)
                    yh = yh_p.tile([128, D], F32, name="yh", tag="yh")
                    nc.vector.tensor_add(yh[:], ps[:].bitcast(F32), st["ht"][:, tb].bitcast(F32))
                    nc.sync.dma_start(
                        out=ho[tok0 + tb * 128: tok0 + (tb + 1) * 128, :], in_=yh[:])

            def proj_mm(t):
                """proj einsum matmuls (token-major) -> p psum tiles."""
                st = TOK[t]
                ps_list = []
                for d in range(2):
                    for tb in range(2):
                        for mi in range(MS[d]):
                            psp = ps_work.tile([128, D], F32, name="work", tag="work")
                            for dc in range(2):
                                nc.tensor.matmul(
                                    psp[:],
                                    st["xc"][d][:, dc, tb * 128:(tb + 1) * 128, mi],
                                    wproj[d][dc][:],
                                    start=(dc == 0), stop=(dc == 1))
                            ps_list.append(psp)
                        yield
                st["p_ps"] = ps_list

            def proj_mod(t):
                """gate p by m2T, add into xt in place, store y."""
                st = TOK[t]
                tok0 = t * T
                k = 0
                for d in range(2):
                    for tb in range(2):
                        xtv = st["xt"][d][:, tb].rearrange("p (d m) -> p d m", m=MS[d])
                        for mi in range(MS[d]):
                            psp = st["p_ps"][k]; k += 1
                            mt = mod_p.tile([128, D], F32, name="modtmp", tag="modtmp")
                            nc.vector.tensor_mul(mt[:], psp[:], st["m2Ts"][d][tb][:])
                            nc.gpsimd.tensor_add(
                                xtv[:, :, mi],
                                xtv[:, :, mi].bitcast(F32), mt[:])
                        out_ap = (y1, y2)[d]
                        sl = slice(tok0 + tb * 128, tok0 + (tb + 1) * 128)
                        nc.sync.dma_start(
                            out=out_ap[sl].rearrange("n d m -> n (d m)"),
                            in_=st["xt"][d][:, tb].bitcast(F32))
                        yield

            def interleave(*gens):
                gens = [g for g in gens if g is not None]
                while gens:
                    nxt = []
                    for g_ in gens:
                        try:
                            next(g_)
                            nxt.append(g_)
                        except StopIteration:
                            pass
                    gens = nxt

            def chain(*gens):
                for g_ in gens:
                    yield from g_

            def drain(g_):
                for _ in g_:
                    pass

            # software-pipelined emission: MAC-dense matmul streams are
            # interleaved with the (LDW-heavy) transpose streams so the PE
            # activity monitor keeps the clock at full rate.
            pending = None  # un-drained generator from previous group
            drain(chain(load_d0(0), load_d1(0)))
            for g in range(n_groups):
                t0, t1 = g * GROUP, g * GROUP + 1
                phase_sqrt(t0)
                act_preload_silu(TOK[t0]["inv"][1][1])
                interleave(chain(load_d0(t1), load_d1(t1)),
                           phase_mlp(t0))
                phase_sqrt(t1)
                act_preload_silu(TOK[t1]["inv"][1][1])
                interleave(phase_mlp(t1), proj_mm(t0))
                nxt = chain(load_d0(t1 + 1), load_d1(t1 + 1)) \
                    if g + 1 < n_groups else None
                interleave(chain(proj_mod(t0), proj_mm(t1), proj_mod(t1)), nxt)
                del TOK[t0], TOK[t1]

    nc.compile()
    return nc


_NC_CACHE = {}


def _get_nc(n_tokens):
    if n_tokens not in _NC_CACHE:
        _NC_CACHE[n_tokens] = build_kernel(n_tokens)
    return _NC_CACHE[n_tokens]


def kernel(h, x1, x2, proj1, proj2, pre1, pre2,
           w1_0, b1_0, w2_0, b2_0, w1_1, b1_1, w2_1, b2_1):
    h = np.asarray(h, np.float32)
    x1 = np.asarray(x1, np.float32)
    x2 = np.asarray(x2, np.float32)
    n_tokens = h.shape[1]
    nc = _get_nc(n_tokens)
    weights = dict(
        pre1=np.asarray(pre1, np.float32), pre2=np.asarray(pre2, np.float32),
        proj1=np.asarray(proj1, np.float32), proj2=np.asarray(proj2, np.float32),
        w1_0=np.asarray(w1_0, np.float32), b1_0=np.asarray(b1_0, np.float32),
        w2_0=np.asarray(w2_0, np.float32), b2_0=np.asarray(b2_0, np.float32),
        w1_1=np.asarray(w1_1, np.float32), b1_1=np.asarray(b1_1, np.float32),
        w2_1=np.asarray(w2_1, np.float32), b2_1=np.asarray(b2_1, np.float32),
    )
    n_cores = h.shape[0]
    in_maps = [dict(h=h[c], x1=x1[c], x2=x2[c], **weights) for c in range(n_cores)]
    res = run_bass_kernel_spmd(nc, in_maps, list(range(n_cores)))
    h_new = np.stack([res.results[c]["ho"] for c in range(n_cores)])
    y1 = np.stack([res.results[c]["y1"] for c in range(n_cores)])
    y2 = np.stack([res.results[c]["y2"] for c in range(n_cores)])
    return h_new, y1, y2


if __name__ == "__main__":
    rng = np.random.default_rng(0)
    nt = 512
    inputs = dict(
        h=rng.standard_normal((1, nt, D), dtype=np.float32),
        x1=rng.standard_normal((1, nt, D, M1), dtype=np.float32),
        x2=rng.standard_normal((1, nt, D, M2), dtype=np.float32),
        pre1=rng.standard_normal((D, D), dtype=np.float32),
        pre2=rng.standard_normal((D, D), dtype=np.float32),
        proj1=rng.standard_normal((D, D), dtype=np.float32),
        proj2=rng.standard_normal((D, D), dtype=np.float32),
        w1_0=(rng.standard_normal((K2D, HID)) * 0.02).astype(np.float32),
        b1_0=np.zeros(HID, np.float32),
        w2_0=(rng.standard_normal((HID, K2D)) * 0.02).astype(np.float32),
        b2_0=np.zeros(K2D, np.float32),
        w1_1=(rng.standard_normal((K2D, HID)) * 0.02).astype(np.float32),
        b1_1=np.zeros(HID, np.float32),
        w2_1=(rng.standard_normal((HID, K2D)) * 0.02).astype(np.float32),
        b2_1=np.zeros(K2D, np.float32),
    )

    def ref_one(h, x, pre, proj, w1, b1, w2, b2):
        pm = np.einsum('ndm,de->nem', x, pre)
        inv = np.sqrt((pm * pm).sum(-1))
        mi = np.concatenate([h, inv], -1)
        z = mi @ w1 + b1
        hid = z / (1 + np.exp(-z))
        out = hid @ w2 + b2
        m1, m2 = out[:, :D], out[:, D:]
        p = np.einsum('ndm,de->nem', x, proj)
        return m1, p * m2[:, :, None]

    m1a, mod1 = ref_one(inputs['h'][0], inputs['x1'][0], inputs['pre1'],
                        inputs['proj1'], inputs['w1_0'], inputs['b1_0'],
                        inputs['w2_0'], inputs['b2_0'])
    m1b, mod2 = ref_one(inputs['h'][0], inputs['x2'][0], inputs['pre2'],
                        inputs['proj2'], inputs['w1_1'], inputs['b1_1'],
                        inputs['w2_1'], inputs['b2_1'])
    eh = inputs['h'][0] + m1a + m1b
    e1 = inputs['x1'][0] + mod1
    e2 = inputs['x2'][0] + mod2

    ho_, y1_, y2_ = kernel(**inputs)
    for name, got, want in [("h", ho_[0], eh), ("y1", y1_[0], e1), ("y2", y2_[0], e2)]:
        err = np.abs(got - want).max()
        rel = np.linalg.norm(got - want) / np.linalg.norm(want)
        print(f"{name}: absmax {err:.3e} rel {rel:.3e}")
